# revision 1
# baseline (speedup 1.0000x reference)
"""LIF (leaky integrate-and-fire) recurrence kernel for Trainium2, 8 NeuronCores.

Problem: x (T=32, B=64, N=32768) f32.
    m[t] = tau*v[t-1] + x[t];  y[t] = (m[t] >= 1.0);  v[t] = m[t]*(1-y[t])
Output: y (32, 64, 32768) f32.

Sharding: data-parallel over batch. Core c handles x[:, 8c:8(c+1), :],
a (32, 262144)-element independent recurrence.

Per-core pipeline (bit-exact vs the f32 reference):
  DVE (two fused scalar_tensor_tensor ops per step):
    m = (v * tau) + x_t            (in0 op0 scalar) op1 in1
    v = (m is_lt 1.0) * m          hard reset: v=m below threshold, else 0
  ACT (spike output, exact at the threshold):
    s = Sign(m - 1)                m-1 is exact (Sterbenz), s in {-1,0,+1}
    y = Sigmoid(1e4*s + 5e3)       saturates: s=-1 -> 0.0, s in {0,+1} -> 1.0
                                   (m == 1.0 exactly gives y = 1, matching the
                                   reference's u >= 0)
x loads are staged [1,3,4,4,...] timesteps (fast pipeline fill) on the sync
HWDGE ring; y stores flush every 2 timesteps (short drain) on the scalar ring
as bf16 (0/1 is exact in bf16; host widens back to f32), halving write traffic.

Measured (core 0 NTFF): ~175.7 us at full clock (the part sometimes runs a
0.8 GHz DVFS mode where the same NEFF takes ~208 us). DVE floor for the 64
scalar_tensor_tensor ops alone is 146.6 us; rel err vs the f32 reference is 0.
"""

import sys

if "/opt/trn_rl_repo" not in sys.path:
    sys.path.insert(0, "/opt/trn_rl_repo")

import numpy as np

TAU = 0.5
V_TH = 1.0

N_CORES = 8
T, B, N = 32, 64, 32768
B_SH = B // N_CORES          # 8 batch rows per core
E = B_SH * N                 # 262144 elements per core per timestep
P = 128                      # SBUF partitions
F = E // P                   # 2048 f32 per partition per timestep

X_CHUNKS = [1, 3] + [4] * 7            # timesteps per x load (fast fill)
Y_CHUNKS = [2] * 15 + [1, 1]           # timesteps per y store (fast drain)
# last k timesteps compute the spike on DVE (tensor_scalar) instead of the
# ACT Sign/Sigmoid chain, so the drain doesn't wait for ACT
LAST_DVE_STEPS = 1
Y_BF16 = True                          # y is exactly 0/1: store bf16, halve writes

_compiled = None


def _build():
    from concourse import bacc, tile, mybir

    f32 = mybir.dt.float32
    ydt = mybir.dt.bfloat16 if Y_BF16 else f32
    assert sum(X_CHUNKS) == T and sum(Y_CHUNKS) == T
    nc = bacc.Bacc("TRN2", debug=False, num_devices=N_CORES)
    x = nc.dram_tensor("x", [T, E], f32, kind="ExternalInput").ap()
    y = nc.dram_tensor("y", [T, E], ydt, kind="ExternalOutput").ap()

    # [t, p, f] views of DRAM
    x_r = x.rearrange("t (p f) -> t p f", p=P)
    y_r = y.rearrange("t (p f) -> t p f", p=P)

    with tile.TileContext(nc) as tc:
        with (
            tc.tile_pool(name="io", bufs=3) as io_pool,
            tc.tile_pool(name="state", bufs=1) as st_pool,
            tc.tile_pool(name="m", bufs=5) as m_pool,
            tc.tile_pool(name="s", bufs=3) as s_pool,
            tc.tile_pool(name="yp", bufs=3) as y_pool,
        ):
            # per-partition constants for the ACT affine args
            c_neg1 = st_pool.tile([P, 1], f32, tag="c_neg1")
            c_scale = st_pool.tile([P, 1], f32, tag="c_scale")
            c_bias = st_pool.tile([P, 1], f32, tag="c_bias")
            nc.gpsimd.memset(c_neg1[:], -V_TH)
            nc.gpsimd.memset(c_scale[:], 1.0e4)
            nc.gpsimd.memset(c_bias[:], 5.0e3)
            v = st_pool.tile([P, F], f32, tag="v")
            nc.gpsimd.memset(v[:], 0.0)

            # issue x loads lazily, two chunks ahead of consumption
            x_tiles = {}          # t -> (tile, col offset)
            next_chunk = 0
            t_loaded = 0

            def load_chunk():
                nonlocal next_chunk, t_loaded
                n_t = X_CHUNKS[next_chunk]
                xt = io_pool.tile([P, 4 * F], f32, tag="x")
                nc.sync.dma_start(
                    out=xt[:, : n_t * F].rearrange("p (t f) -> p t f", t=n_t),
                    in_=x_r[t_loaded:t_loaded + n_t].rearrange("t p f -> p t f"),
                )
                for i in range(n_t):
                    x_tiles[t_loaded + i] = (xt, i * F)
                next_chunk += 1
                t_loaded += n_t

            load_chunk()
            y_t = None
            y_chunk_idx = 0
            y_off = 0  # timesteps into current y chunk
            for t in range(T):
                if t not in x_tiles:
                    load_chunk()
                if next_chunk < len(X_CHUNKS) and t == t_loaded - X_CHUNKS[next_chunk - 1]:
                    load_chunk()  # prefetch one chunk ahead
                xt, off = x_tiles.pop(t)
                xs = xt[:, off:off + F]
                n_yt = Y_CHUNKS[y_chunk_idx]
                if y_off == 0:
                    y_t = y_pool.tile([P, max(Y_CHUNKS) * F], ydt, tag="y")
                ys = y_t[:, y_off * F:(y_off + 1) * F]
                m = m_pool.tile([P, F], f32, tag="m")
                # m = (v * tau) + x_t
                nc.vector.scalar_tensor_tensor(
                    out=m[:], in0=v[:], scalar=TAU, in1=xs,
                    op0=mybir.AluOpType.mult, op1=mybir.AluOpType.add,
                )
                # v = (m < vth) * m   (hard reset)
                nc.vector.scalar_tensor_tensor(
                    out=v[:], in0=m[:], scalar=V_TH, in1=m[:],
                    op0=mybir.AluOpType.is_lt, op1=mybir.AluOpType.mult,
                )
                if t >= T - LAST_DVE_STEPS:
                    # last step: spike on DVE so the final store doesn't wait
                    # for the ACT chain: y = (m is_ge 1) * 1
                    nc.vector.tensor_scalar(
                        out=ys, in0=m[:], scalar1=V_TH, scalar2=1.0,
                        op0=mybir.AluOpType.is_ge, op1=mybir.AluOpType.mult,
                    )
                else:
                    # s = Sign(m - 1); y = Sigmoid(1e4*s + 5e3)
                    s = s_pool.tile([P, F], f32, tag="s")
                    nc.scalar.activation(
                        out=s[:], in_=m[:],
                        func=mybir.ActivationFunctionType.Sign,
                        bias=c_neg1[:], scale=1.0,
                    )
                    nc.scalar.activation(
                        out=ys, in_=s[:],
                        func=mybir.ActivationFunctionType.Sigmoid,
                        bias=c_bias[:], scale=c_scale[:],
                    )
                y_off += 1
                if y_off == n_yt:
                    nc.scalar.dma_start(
                        out=y_r[t - n_yt + 1:t + 1].rearrange("t p f -> p t f"),
                        in_=y_t[:, : n_yt * F].rearrange("p (t f) -> p t f", t=n_yt),
                    )
                    y_chunk_idx += 1
                    y_off = 0
    nc.compile()
    return nc


def _get_compiled():
    global _compiled
    if _compiled is None:
        _compiled = _build()
        # warm the NEFF (first execution pays ~20us of cold-start)
        import concourse.bass_utils as bass_utils

        z = [{"x": np.zeros((T, E), dtype=np.float32)} for _ in range(N_CORES)]
        bass_utils.run_bass_kernel_spmd(
            _compiled, z, core_ids=list(range(N_CORES))
        )
    return _compiled


def kernel(x: np.ndarray, _trace: bool = False):
    import concourse.bass_utils as bass_utils

    nc = _get_compiled()
    x = np.ascontiguousarray(x, dtype=np.float32)
    in_maps = [
        {"x": x[:, c * B_SH:(c + 1) * B_SH, :].reshape(T, E)}
        for c in range(N_CORES)
    ]
    res = bass_utils.run_bass_kernel_spmd(
        nc, in_maps, core_ids=list(range(N_CORES)), trace=_trace
    )
    y = np.empty((T, B, N), dtype=np.float32)
    for c in range(N_CORES):
        yc = res.results[c]["y"]
        if yc.dtype != np.float32:
            yc = yc.astype(np.float32)  # bf16 0/1 -> f32, exact
        y[:, c * B_SH:(c + 1) * B_SH, :] = yc.reshape(T, B_SH, N)
    if _trace:
        return y, res
    return y



# revision 2
# speedup vs baseline: 1.2931x; 1.2931x over previous
"""LIF (leaky integrate-and-fire) recurrence kernel for Trainium2, 8 NeuronCores.

Problem: x (T=32, B=64, N=32768) f32.
    m[t] = tau*v[t-1] + x[t];  y[t] = (m[t] >= 1.0);  v[t] = m[t]*(1-y[t])
Output: y (32, 64, 32768) f32.

Sharding: data-parallel over batch. Core c handles x[:, 8c:8(c+1), :],
a (32, 262144)-element independent recurrence.

Per-core pipeline (bit-exact vs the f32 reference):
  DVE — ONE custom-DVE op per timestep (registered at import, the
  supported dve_ops.OPS extension path). State is m (not v); the reset
  folds into the next step's read:
      m[t] = select(m[t-1] < vth, m[t-1], 0) * tau + x[t]
  4 ALU stages, 1 elem/cycle -> ~2.2us/step instead of the 2-op
  scalar_tensor_tensor chain (~4.6us/step).
  ACT — y in ONE op: s = Sign(m - c) with c = 1 - 2^-24 (the f32 just
  below vth). s = +1  <=>  m >= vth exactly (Sterbenz), s in {-1,0,+1}
  stored as int8; the host maps y = (s == 1). int8 quarters the y DMA
  traffic vs bf16.
x loads are staged [1,3,4,4,...] timesteps (fast pipeline fill) on the
sync HWDGE ring; y stores flush every 4 timesteps (short drain at the
tail) on the scalar ring.

With DVE ~70us and ACT ~61us busy, the kernel is DMA-bound: 33.55MB x in
+ 8.39MB y out = 41.9MB/core at ~360-400GB/s -> ~105-117us.
"""

import sys

if "/opt/trn_rl_repo" not in sys.path:
    sys.path.insert(0, "/opt/trn_rl_repo")

import numpy as np

TAU = 0.5
V_TH = 1.0
C_TH = 1.0 - 2.0 ** -24      # largest f32 < V_TH

N_CORES = 8
T, B, N = 32, 64, 32768
B_SH = B // N_CORES          # 8 batch rows per core
E = B_SH * N                 # 262144 elements per core per timestep
P = 128                      # SBUF partitions
F = E // P                   # 2048 f32 per partition per timestep

X_CHUNKS = [1, 3] + [4] * 7            # timesteps per x load (fast fill)
Y_CHUNKS = [4] * 7 + [2, 1, 1]         # timesteps per y store (fast drain)

_compiled = None


def _register_lif_op():
    """Register the fused LIF step as a custom DVE op (dve_ops.OPS append,
    the documented extension path; the uop table is generated per-NEFF)."""
    import concourse.dve_ops as dve_ops
    from concourse.dve_spec import (
        Spec, Src0, Src1, C0, C1, Zero, select, lower, _has_src1,
    )
    from concourse.dve_uop import DveOpSpec

    name = "LIF_STEP_ANT"
    for op in dve_ops.OPS:
        if op.name == name:
            return op
    body = select(Src0 < C1, Src0, Zero) * C0 + Src1
    spec = Spec(
        body=body,
        reference=lambda in0, in1, s0, s1, imm2: (
            np.where(in0 < s1, in0, np.float32(0.0)).astype(np.float32)
            * np.float32(s0) + in1
        ).astype(np.float32),
    )
    row = dve_ops._CUSTOM_DVE_ROW_BASE + len(dve_ops.OPS)
    assert row < 0x20
    dve_ops._SUB_OPCODE_FOR_NAME[name] = row
    sha = DveOpSpec(
        name=name, uops=lower(spec, ver="v3"), rd1_en=_has_src1(spec)
    ).sha("v3")
    op = dve_ops.DveOp(name, spec, subdim=False, uops_sha={"v3": sha})
    dve_ops.OPS.append(op)
    dve_ops.CUSTOM_DVE_SPECS[name] = spec
    return op


def _build():
    from concourse import bacc, tile, mybir

    lif_op = _register_lif_op()
    f32 = mybir.dt.float32
    i8 = mybir.dt.int8
    assert sum(X_CHUNKS) == T and sum(Y_CHUNKS) == T
    nc = bacc.Bacc("TRN2", debug=False, num_devices=N_CORES)
    x = nc.dram_tensor("x", [T, E], f32, kind="ExternalInput").ap()
    y = nc.dram_tensor("y", [T, E], i8, kind="ExternalOutput").ap()

    # [t, p, f] views of DRAM
    x_r = x.rearrange("t (p f) -> t p f", p=P)
    y_r = y.rearrange("t (p f) -> t p f", p=P)

    with tile.TileContext(nc) as tc:
        with (
            tc.tile_pool(name="io", bufs=3) as io_pool,
            tc.tile_pool(name="state", bufs=1) as st_pool,
            tc.tile_pool(name="m", bufs=4) as m_pool,
            tc.tile_pool(name="yp", bufs=3) as y_pool,
        ):
            cb = st_pool.tile([P, 1], f32, tag="cb")     # Sign bias = -C_TH
            nc.gpsimd.memset(cb[:], -C_TH)
            m_prev = st_pool.tile([P, F], f32, tag="m0")  # v[-1] = 0 seed
            nc.gpsimd.memset(m_prev[:], 0.0)

            # issue x loads lazily, two chunks ahead of consumption
            x_tiles = {}          # t -> (tile, col offset)
            next_chunk = 0
            t_loaded = 0

            def load_chunk():
                nonlocal next_chunk, t_loaded
                n_t = X_CHUNKS[next_chunk]
                xt = io_pool.tile([P, 4 * F], f32, tag="x")
                nc.sync.dma_start(
                    out=xt[:, : n_t * F].rearrange("p (t f) -> p t f", t=n_t),
                    in_=x_r[t_loaded:t_loaded + n_t].rearrange("t p f -> p t f"),
                )
                for i in range(n_t):
                    x_tiles[t_loaded + i] = (xt, i * F)
                next_chunk += 1
                t_loaded += n_t

            load_chunk()
            y_t = None
            y_chunk_idx = 0
            y_off = 0  # timesteps into current y chunk
            for t in range(T):
                if t not in x_tiles:
                    load_chunk()
                if next_chunk < len(X_CHUNKS) and t == t_loaded - X_CHUNKS[next_chunk - 1]:
                    load_chunk()  # prefetch one chunk ahead
                xt, off = x_tiles.pop(t)
                xs = xt[:, off:off + F]
                n_yt = Y_CHUNKS[y_chunk_idx]
                if y_off == 0:
                    y_t = y_pool.tile([P, max(Y_CHUNKS) * F], i8, tag="y")
                ys = y_t[:, y_off * F:(y_off + 1) * F]
                m = m_pool.tile([P, F], f32, tag="m")
                # m = select(m_prev < vth, m_prev, 0) * tau + x_t
                nc.vector._custom_dve(
                    lif_op, out=m[:], in0=m_prev[:], in1=xs, s0=TAU, s1=V_TH,
                )
                # s = Sign(m - c) -> int8; +1 iff m >= vth
                nc.scalar.activation(
                    out=ys, in_=m[:],
                    func=mybir.ActivationFunctionType.Sign,
                    bias=cb[:], scale=1.0,
                )
                m_prev = m
                y_off += 1
                if y_off == n_yt:
                    nc.scalar.dma_start(
                        out=y_r[t - n_yt + 1:t + 1].rearrange("t p f -> p t f"),
                        in_=y_t[:, : n_yt * F].rearrange("p (t f) -> p t f", t=n_yt),
                    )
                    y_chunk_idx += 1
                    y_off = 0
    nc.compile()
    return nc


def _get_compiled():
    global _compiled
    if _compiled is None:
        _compiled = _build()
        # warm the NEFF (first execution pays ~20us of cold-start)
        import concourse.bass_utils as bass_utils

        z = [{"x": np.zeros((T, E), dtype=np.float32)} for _ in range(N_CORES)]
        bass_utils.run_bass_kernel_spmd(
            _compiled, z, core_ids=list(range(N_CORES))
        )
    return _compiled


def kernel(x: np.ndarray, _trace: bool = False):
    import concourse.bass_utils as bass_utils

    nc = _get_compiled()
    x = np.ascontiguousarray(x, dtype=np.float32)
    in_maps = [
        {"x": x[:, c * B_SH:(c + 1) * B_SH, :].reshape(T, E)}
        for c in range(N_CORES)
    ]
    res = bass_utils.run_bass_kernel_spmd(
        nc, in_maps, core_ids=list(range(N_CORES)), trace=_trace
    )
    y = np.empty((T, B, N), dtype=np.float32)
    for c in range(N_CORES):
        sc = res.results[c]["y"]          # int8 in {-1, 0, +1}
        y[:, c * B_SH:(c + 1) * B_SH, :] = (sc.reshape(T, B_SH, N) == 1)
    if _trace:
        return y, res
    return y


# revision 4
# speedup vs baseline: 1.3259x; 1.0254x over previous
"""LIF (leaky integrate-and-fire) recurrence kernel for Trainium2, 8 NeuronCores.

Problem: x (T=32, B=64, N=32768) f32.
    m[t] = tau*v[t-1] + x[t];  y[t] = (m[t] >= 1.0);  v[t] = m[t]*(1-y[t])
Output: y (32, 64, 32768) f32.

Sharding: data-parallel over batch. Core c handles x[:, 8c:8(c+1), :],
a (32, 262144)-element independent recurrence.

Per-core pipeline (bit-exact vs the f32 reference):
  DVE — ONE custom-DVE op per timestep (registered at import, the
  supported dve_ops.OPS extension path). State is m (not v); the reset
  folds into the next step's read:
      m[t] = select(m[t-1] < vth, m[t-1], 0) * tau + x[t]
  4 ALU stages, 1 elem/cycle -> ~2.2us/step instead of the 2-op
  scalar_tensor_tensor chain (~4.6us/step).
  ACT — y in ONE op: s = Sign(m - c) with c = 1 - 2^-24 (the f32 just
  below vth). s = +1  <=>  m >= vth exactly (Sterbenz), s in {-1,0,+1}
  stored as int8; the host maps y = (s == 1). int8 quarters the y DMA
  traffic vs bf16.
x loads are staged [1,3,4,4,...] timesteps (fast pipeline fill) on the
sync HWDGE ring; y stores flush every 4 timesteps (short drain at the
tail) on the scalar ring.

With DVE ~70us and ACT ~61us busy, the kernel is DMA-bound: 33.55MB x in
+ 8.39MB y out = 41.9MB/core at ~360-400GB/s -> ~105-117us.
"""

import sys

if "/opt/trn_rl_repo" not in sys.path:
    sys.path.insert(0, "/opt/trn_rl_repo")

import numpy as np

TAU = 0.5
V_TH = 1.0
C_TH = 1.0 - 2.0 ** -24      # largest f32 < V_TH

N_CORES = 8
T, B, N = 32, 64, 32768
B_SH = B // N_CORES          # 8 batch rows per core
E = B_SH * N                 # 262144 elements per core per timestep
P = 128                      # SBUF partitions
F = E // P                   # 2048 f32 per partition per timestep

X_CHUNKS = [1, 3] + [4] * 6 + [2, 1, 1]  # timesteps per x load (fast fill+drain)
Y_CHUNKS = [4] * 7 + [2, 1, 1]           # timesteps per y store (fast drain)

_compiled = None


def _register_lif_op():
    """Register the fused LIF step as a custom DVE op (dve_ops.OPS append,
    the documented extension path; the uop table is generated per-NEFF)."""
    import concourse.dve_ops as dve_ops
    from concourse.dve_spec import (
        Spec, Src0, Src1, C0, C1, Zero, select, lower, _has_src1,
    )
    from concourse.dve_uop import DveOpSpec

    name = "LIF_STEP_ANT"
    for op in dve_ops.OPS:
        if op.name == name:
            return op
    body = select(Src0 < C1, Src0, Zero) * C0 + Src1
    spec = Spec(
        body=body,
        reference=lambda in0, in1, s0, s1, imm2: (
            np.where(in0 < s1, in0, np.float32(0.0)).astype(np.float32)
            * np.float32(s0) + in1
        ).astype(np.float32),
    )
    row = dve_ops._CUSTOM_DVE_ROW_BASE + len(dve_ops.OPS)
    assert row < 0x20
    dve_ops._SUB_OPCODE_FOR_NAME[name] = row
    sha = DveOpSpec(
        name=name, uops=lower(spec, ver="v3"), rd1_en=_has_src1(spec)
    ).sha("v3")
    op = dve_ops.DveOp(name, spec, subdim=False, uops_sha={"v3": sha})
    dve_ops.OPS.append(op)
    dve_ops.CUSTOM_DVE_SPECS[name] = spec
    return op


def _build():
    from concourse import bacc, tile, mybir

    lif_op = _register_lif_op()
    f32 = mybir.dt.float32
    i8 = mybir.dt.int8
    assert sum(X_CHUNKS) == T and sum(Y_CHUNKS) == T
    nc = bacc.Bacc("TRN2", debug=False, num_devices=N_CORES)
    x = nc.dram_tensor("x", [T, E], f32, kind="ExternalInput").ap()
    y = nc.dram_tensor("y", [T, E], i8, kind="ExternalOutput").ap()

    # [t, p, f] views of DRAM
    x_r = x.rearrange("t (p f) -> t p f", p=P)
    y_r = y.rearrange("t (p f) -> t p f", p=P)

    with tile.TileContext(nc) as tc:
        with (
            tc.tile_pool(name="io", bufs=3) as io_pool,
            tc.tile_pool(name="state", bufs=1) as st_pool,
            tc.tile_pool(name="m", bufs=6) as m_pool,
            tc.tile_pool(name="yp", bufs=4) as y_pool,
        ):
            cb = st_pool.tile([P, 1], f32, tag="cb")     # Sign bias = -C_TH
            nc.gpsimd.memset(cb[:], -C_TH)
            m_prev = st_pool.tile([P, F], f32, tag="m0")  # v[-1] = 0 seed
            nc.gpsimd.memset(m_prev[:], 0.0)

            # issue x loads lazily, two chunks ahead of consumption
            x_tiles = {}          # t -> (tile, col offset)
            next_chunk = 0
            t_loaded = 0

            def load_chunk():
                nonlocal next_chunk, t_loaded
                n_t = X_CHUNKS[next_chunk]
                xt = io_pool.tile([P, 4 * F], f32, tag="x")
                nc.sync.dma_start(
                    out=xt[:, : n_t * F].rearrange("p (t f) -> p t f", t=n_t),
                    in_=x_r[t_loaded:t_loaded + n_t].rearrange("t p f -> p t f"),
                )
                for i in range(n_t):
                    x_tiles[t_loaded + i] = (xt, i * F)
                next_chunk += 1
                t_loaded += n_t

            load_chunk()
            y_t = None
            y_chunk_idx = 0
            y_off = 0  # timesteps into current y chunk
            for t in range(T):
                if t not in x_tiles:
                    load_chunk()
                if next_chunk < len(X_CHUNKS) and t == t_loaded - X_CHUNKS[next_chunk - 1]:
                    load_chunk()  # prefetch one chunk ahead
                xt, off = x_tiles.pop(t)
                xs = xt[:, off:off + F]
                n_yt = Y_CHUNKS[y_chunk_idx]
                if y_off == 0:
                    y_t = y_pool.tile([P, max(Y_CHUNKS) * F], i8, tag="y")
                ys = y_t[:, y_off * F:(y_off + 1) * F]
                m = m_pool.tile([P, F], f32, tag="m")
                # m = select(m_prev < vth, m_prev, 0) * tau + x_t
                nc.vector._custom_dve(
                    lif_op, out=m[:], in0=m_prev[:], in1=xs, s0=TAU, s1=V_TH,
                )
                # s = Sign(m - c) -> int8; +1 iff m >= vth
                nc.scalar.activation(
                    out=ys, in_=m[:],
                    func=mybir.ActivationFunctionType.Sign,
                    bias=cb[:], scale=1.0,
                )
                m_prev = m
                y_off += 1
                if y_off == n_yt:
                    nc.scalar.dma_start(
                        out=y_r[t - n_yt + 1:t + 1].rearrange("t p f -> p t f"),
                        in_=y_t[:, : n_yt * F].rearrange("p (t f) -> p t f", t=n_yt),
                    )
                    y_chunk_idx += 1
                    y_off = 0
    nc.compile()
    return nc


def _get_compiled():
    global _compiled
    if _compiled is None:
        _compiled = _build()
        # warm the NEFF (first execution pays ~20us of cold-start)
        import concourse.bass_utils as bass_utils

        z = [{"x": np.zeros((T, E), dtype=np.float32)} for _ in range(N_CORES)]
        bass_utils.run_bass_kernel_spmd(
            _compiled, z, core_ids=list(range(N_CORES))
        )
    return _compiled


def kernel(x: np.ndarray, _trace: bool = False):
    import concourse.bass_utils as bass_utils

    nc = _get_compiled()
    x = np.ascontiguousarray(x, dtype=np.float32)
    in_maps = [
        {"x": x[:, c * B_SH:(c + 1) * B_SH, :].reshape(T, E)}
        for c in range(N_CORES)
    ]
    res = bass_utils.run_bass_kernel_spmd(
        nc, in_maps, core_ids=list(range(N_CORES)), trace=_trace
    )
    y = np.empty((T, B, N), dtype=np.float32)
    for c in range(N_CORES):
        sc = res.results[c]["y"]          # int8 in {-1, 0, +1}
        y[:, c * B_SH:(c + 1) * B_SH, :] = (sc.reshape(T, B_SH, N) == 1)
    if _trace:
        return y, res
    return y
